# revision 18
# baseline (speedup 1.0000x reference)
"""Bass/Trainium2 kernel for nn_Attentioncell (Bahdanau-style attention cell).

Mathematical simplification (rel-err ~6e-7 vs the jax reference): the
per-step scores are
    scores[b,l] = (total[b,l,:] + (h @ W2)[b,:]) @ V
               = (total @ V)[b,l] + (h @ W2 @ V)[b]
and softmax over l is invariant to the per-b shift, so the attention
weights are identical for every timestep and independent of h:
    attn = softmax_l(x_static @ (W1 @ V))        (b2, W2, h0 drop out)
    ctx[b,:] = sum_l attn[b,l] * x_static[b,l,:]
    out[b,t,:] = x[b,t,:] @ W3[:D] + ctx[b,:] @ W3[D:] + b3

Additional host-side weight folding: with w1v = W1@V,
    scores[l] = sum_s (x_static[l,s] * w1v[s])   = rowsum(xsw)
    ctx' = E^T @ xsw        (xsw = x_static * w1v, elementwise over s)
    ctx' @ (W3[D:] / w1v)  == ctx @ W3[D:]       (exact algebra)
so the device never multiplies by w1v: scores are plain row-sums of the
pre-scaled xsw, and W3bot is divided by w1v on the host.  Each product
term in the c2 GEMM has exactly the same magnitude as before, so the
bf16 error profile is unchanged (min |w1v| ~ 7e-5 -> max |W3bot'| ~2e3,
comfortably inside bf16 range).

Data-parallel over batch B=32 across 8 NeuronCores (4 per core).

Schedule notes (from perfetto trace analysis of the previous version):
  - PE has a DVFS p-state ramp (0.65 -> 1.2 -> 2.4 GHz after ~3us of
    continuous busy).  Dummy matmuls on a memset tile keep the PE busy
    through the ~2.6us DMA-latency head so real matmuls run at high
    clock; extra dummies are placed in known schedule bubbles.
  - 4 DMA queues (sync/scalar/gpsimd/vector), chunk order chosen by
    consumption deadline; per-DMA latency is ~2.2us fixed (descriptor
    gen + DGE start + completion-semaphore propagation) + transfer.
  - reduces: DVE tensor_reduce for chunks {0,2,4,5,6} (~.85us each),
    ACT Copy+accum for {1,3} (~1.2us each); exps in 4 waves on ACT;
    E-builds (exp * block-diagonal mask) on the otherwise-idle GpSimd.
  - Z = sum_l E via 7 tiny PE matmuls against a ones column riding the
    ctx accumulation; 1/Z is folded into the Ind5 indicator matrix
    (IndC = ind5 * recipZ) so the ctx PSUM->SBUF copy does not wait on
    the reciprocal.
  - out is produced in bf16 (halves the output DMA) and upconverted on
    the host; rel-err stays ~1e-3 vs the 2e-2 gate.
"""

import numpy as np

B, T, L, S, D = 32, 32, 196, 512, 512
NCORES = 8
BLOC = B // NCORES          # 4 batches per core
BT = BLOC * T               # 128 output rows per core
BL = BLOC * L               # 784 static rows per core
NCH = 7                     # xsw chunks
CH = BL // NCH              # 112 rows per chunk
MW = NCH * BLOC             # 28 mask columns

# xsp column layout: [c0 | c1 | mask | c2 | c3 | c4 | c5 | c6]
# (mask rides in the same DMA group as c1/c2 so it lands early for the
# first E-build).
_CHOFF = [0, S, 2 * S + MW, 3 * S + MW, 4 * S + MW, 5 * S + MW, 6 * S + MW]
MASKOFF = 2 * S
XSW = 7 * S + MW            # 3612

# score-column s <-> chunk SCORD[s]: columns ordered by expected DMA
# arrival (c5/c6 ride the GpSimd queue first DMA, c3/c4 land last) so
# reduces/exp waves run in arrival order with contiguous column slices.
SCORD = [0, 1, 2, 5, 6, 3, 4]

# consts layout: [ind5 (128) | id4 (4) | rhs5 region (512)] on 5 partitions.
# rhs5 region row 4 = b3 (via DMA); rows 0..3 overwritten on-device by c2.
CW = 128 + 4 + 512

_cache = {}


def _build_graph():
    import concourse.bacc as bacc
    import concourse.tile as tile
    from concourse import mybir

    f32 = mybir.dt.float32
    bf16 = mybir.dt.bfloat16
    nc = bacc.Bacc("TRN2", target_bir_lowering=False, debug=False,
                   num_devices=NCORES)

    xs_d = nc.dram_tensor("xsp", [CH, XSW], bf16, kind="ExternalInput").ap()
    w3t_d = nc.dram_tensor("w3tx", [128, 512 + 4 * D], bf16,
                           kind="ExternalInput").ap()
    w3b_d = nc.dram_tensor("w3b", [128, 4 * D], bf16, kind="ExternalInput").ap()
    cst_d = nc.dram_tensor("cst", [5, CW], bf16, kind="ExternalInput").ap()
    out_d = nc.dram_tensor("out", [BT, D], bf16, kind="ExternalOutput").ap()

    with tile.TileContext(nc) as tc:
        with (
            tc.tile_pool(name="big", bufs=1) as big,
            tc.tile_pool(name="small", bufs=1) as small,
            tc.tile_pool(name="scratch", bufs=2) as scratch,
            tc.tile_pool(name="ps_acc", bufs=1, space="PSUM") as ps_acc,
            tc.tile_pool(name="ps_tr", bufs=2, space="PSUM") as ps_tr,
        ):
            xsp = big.tile([CH, XSW], bf16, tag="xsp")
            mask = xsp[:, MASKOFF:MASKOFF + MW]
            w3tx = big.tile([128, 512 + 4 * D], bf16, tag="w3tx")
            xt = w3tx[:, 0:512]
            w3t = w3tx[:, 512:]
            w3b = big.tile([128, 4 * D], bf16, tag="w3b")
            cst = small.tile([5, CW], bf16, tag="cst")
            ind5 = cst[:, 0:128]
            id4 = cst[0:4, 128:132]
            rhs5 = cst[:, 132:132 + D]
            dummy = big.tile([128, 512], bf16, tag="dummy")
            ones = small.tile([CH, 1], bf16, tag="ones")
            scores = small.tile([CH, NCH], f32, tag="scores")
            etile = small.tile([CH, NCH], bf16, tag="etile")
            E = small.tile([CH, MW], bf16, tag="E")
            recipZ5 = small.tile([5, 1], f32, tag="recipZ5")
            IndC = small.tile([5, BT], bf16, tag="IndC")
            ctx_sb = small.tile([BLOC, S], bf16, tag="ctx_sb")
            ctxT = small.tile([128, 4 * BLOC], bf16, tag="ctxT")
            out_sb = big.tile([BT, D], bf16, tag="out_sb")

            # ---- memsets first (dummy feeds the PE warmup stream; on DVE
            # so the GpSimd DMA queue starts immediately) ----
            nc.vector.memset(dummy[:], 0.0)

            # ---- DMA loads. The 3 queues (SP/ACT/GpSimd) share ~300GB/s
            # of HBM and each queue processes its list serially, so the
            # global arrival order is round-based across queues: round 1-2
            # carry all score chunks, later rounds carry the GEMM weights
            # (w3b before w3t: the c2 chain consumes w3b ~1.5us before the
            # final matmuls need w3t). ----
            def chunk(c):
                return xsp[:, _CHOFF[c]:_CHOFF[c] + S]

            nc.sync.dma_start(xsp[:, 0:S], xs_d[:, 0:S])                 # c0
            nc.scalar.dma_start(xsp[:, S:_CHOFF[2] + S],
                                xs_d[:, S:_CHOFF[2] + S])                # c1+mask+c2
            nc.gpsimd.dma_start(xsp[:, _CHOFF[5]:_CHOFF[5] + 2 * S],
                                xs_d[:, _CHOFF[5]:_CHOFF[5] + 2 * S])    # c5+c6
            nc.sync.dma_start(xsp[:, _CHOFF[3]:_CHOFF[3] + S],
                              xs_d[:, _CHOFF[3]:_CHOFF[3] + S])          # c3
            nc.scalar.dma_start(xsp[:, _CHOFF[4]:_CHOFF[4] + S],
                                xs_d[:, _CHOFF[4]:_CHOFF[4] + S])        # c4
            nc.gpsimd.dma_start(cst[:], cst_d[:])                        # consts
            nc.gpsimd.dma_start(w3b[:, 0:2 * D], w3b_d[:, 0:2 * D])      # w3b01
            nc.scalar.dma_start(w3b[:, 2 * D:], w3b_d[:, 2 * D:])        # w3b23
            nc.sync.dma_start(w3tx[:, 0:1024], w3t_d[:, 0:1024])         # xt+w3t0
            nc.gpsimd.dma_start(w3tx[:, 1024:1024 + D],
                                w3t_d[:, 1024:1024 + D])                 # w3t1
            nc.sync.dma_start(w3tx[:, 1024 + 2 * D:],
                              w3t_d[:, 1024 + 2 * D:])                   # w3t3
            nc.scalar.dma_start(w3tx[:, 1024 + D:1024 + 2 * D],
                                w3t_d[:, 1024 + D:1024 + 2 * D])         # w3t2
            nc.gpsimd.memset(ones[:], 1.0)
            nc.gpsimd.memset(recipZ5[:], 1.0)  # rows 0..3 overwritten by recip

            out_ps = ps_acc.tile([BT, D], f32, tag="out_ps")
            ctx_ps = ps_acc.tile([BLOC, S], f32, tag="ctx_ps")
            z_ps = ps_acc.tile([BLOC, 1], f32, tag="z_ps")
            c2_ps = ps_acc.tile([BLOC, D], f32, tag="c2_ps")
            dm_ps = ps_acc.tile([128, 512], f32, tag="dm_ps")

            # Manual schedule pins (us): the Tile scheduler's DMA model has
            # no HBM-contention term so it predicts arrivals ~3us early,
            # then compiles engine orders / semaphore bindings that stall
            # the real execution.  tile_wait_until pins each op near its
            # measured real time so the compiled per-engine order matches
            # reality; at runtime everything still runs as early as its
            # data semaphores allow.
            def W(t_us):
                return tc.tile_wait_until(t_us / 1000.0)

            # ---- PE warmup: keep the array busy through the DMA head so
            # the p-state ramps to full clock before real work arrives ----
            def dummy_mm(n):
                for i in range(n):
                    nc.tensor.matmul(dm_ps[:], dummy[:, 0:128], dummy[:],
                                     start=True, stop=True,
                                     skip_group_check=True)

            dummy_mm(8)
            # out_ps accumulation-group start: a zero contribution from the
            # memset dummy tile.  The scheduler hoists PSUM group-start
            # matmuls; giving start=True to this dep-free zero matmul keeps
            # the real out_top matmuls (which wait on the late w3t DMAs)
            # from blocking the in-order PE queue.
            nc.tensor.matmul(out_ps[:], dummy[:, 0:128], dummy[:],
                             start=True, stop=False, skip_group_check=True)
            dummy_mm(2)

            # ---- score reduces, in score-column (= arrival) order.
            # scores[:, s] = rowsum(chunk SCORD[s]); DVE does most, ACT
            # two, GpSimd pre-folds c6 into a [112,256] pair-sum. ----
            c6sum = scratch.tile([CH, 256], bf16, tag="c6sum")

            def dve_reduce(s, t):
                with W(t):
                    nc.vector.tensor_reduce(scores[:, s:s + 1],
                                            chunk(SCORD[s]),
                                            axis=mybir.AxisListType.X,
                                            op=mybir.AluOpType.add)

            def act_reduce(s, t):
                dump = scratch.tile([CH, S], bf16, tag="dump")
                with W(t):
                    nc.scalar.activation(dump[:], chunk(SCORD[s]),
                                         mybir.ActivationFunctionType.Copy,
                                         accum_out=scores[:, s:s + 1])

            def exp_wave(a, b, t):
                with W(t):
                    nc.scalar.activation(etile[:, a:b], scores[:, a:b],
                                         mybir.ActivationFunctionType.Exp)

            def ebuild(a, b, t):
                n = b - a
                with W(t):
                    nc.gpsimd.tensor_mul(
                        E[:, a * BLOC:b * BLOC].rearrange(
                            "p (c b) -> p c b", b=BLOC),
                        etile[:, a:b].to_broadcast((CH, n, BLOC)),
                        mask[:, a * BLOC:b * BLOC].rearrange(
                            "p (c b) -> p c b", b=BLOC),
                    )

            def ctx_mm(s, t):
                with W(t):
                    nc.tensor.matmul(ctx_ps[:],
                                     E[:, s * BLOC:(s + 1) * BLOC],
                                     chunk(SCORD[s]), start=(s == 0),
                                     stop=(s == NCH - 1))
                    nc.tensor.matmul(z_ps[:], E[:, s * BLOC:(s + 1) * BLOC],
                                     ones[:], start=(s == 0),
                                     stop=(s == NCH - 1))

            def out_top(j, t):
                with W(t):
                    nc.tensor.matmul(out_ps[:], xt[:, j * 128:(j + 1) * 128],
                                     w3t[:, j * D:(j + 1) * D],
                                     start=False, stop=False,
                                     skip_group_check=True)

            # reduces (score col, pin): DVE cols 0,2,3,4h,6; ACT cols 1,5
            dve_reduce(0, 10.0)
            dve_reduce(2, 11.1)
            dve_reduce(3, 12.2)
            with W(12.2):
                nc.gpsimd.tensor_add(c6sum[:], chunk(6)[:, 0:256],
                                     chunk(6)[:, 256:512])
            with W(12.95):
                nc.vector.tensor_reduce(scores[:, 4:5], c6sum[:],
                                        axis=mybir.AxisListType.X,
                                        op=mybir.AluOpType.add)
            dve_reduce(6, 13.4)
            act_reduce(1, 10.7)
            exp_wave(0, 1, 11.7)
            exp_wave(1, 3, 12.0)
            act_reduce(5, 12.55)
            exp_wave(3, 5, 13.5)
            exp_wave(5, 7, 14.0)
            ebuild(0, 1, 12.0)
            ebuild(1, 3, 12.35)
            ebuild(3, 5, 13.8)
            ebuild(5, 7, 14.3)
            # PE ctx/z stream
            ctx_mm(0, 12.25)
            ctx_mm(1, 12.55)
            ctx_mm(2, 12.85)
            ctx_mm(3, 14.05)
            ctx_mm(4, 14.35)
            ctx_mm(5, 14.55)
            ctx_mm(6, 14.85)

            # ---- 1/Z -> IndC (off the ctx critical path) ----
            with W(15.3):
                nc.vector.reciprocal(recipZ5[0:4, :], z_ps[:])
            with W(15.45):
                nc.vector.tensor_scalar_mul(IndC[:], ind5[:], recipZ5[:])

            # ---- ctx PSUM->SBUF (col-split ACT/DVE), transpose, c2 GEMM ----
            with W(15.4):
                nc.scalar.copy(ctx_sb[:, 0:256], ctx_ps[:, 0:256])
            with W(15.65):
                nc.vector.tensor_copy(ctx_sb[:, 256:512], ctx_ps[:, 256:512])
            for j in range(4):
                tr = ps_tr.tile([128, BLOC], bf16, tag="tr")
                with W(16.05 + 0.08 * j):
                    nc.tensor.transpose(tr[:],
                                        ctx_sb[:, j * 128:(j + 1) * 128],
                                        id4)
                with W(16.15 + 0.12 * j):
                    nc.vector.tensor_copy(ctxT[:, j * BLOC:(j + 1) * BLOC],
                                          tr[:])
            for j in range(4):
                with W(16.35 + 0.17 * j):
                    nc.tensor.matmul(c2_ps[:],
                                     ctxT[:, j * BLOC:(j + 1) * BLOC],
                                     w3b[:, j * D:(j + 1) * D],
                                     start=(j == 0), stop=(j == 3))

            # rhs5 rows 0..3 = unnormalized c2 (split copy ACT/DVE);
            # row 4 = b3 (already there via the consts DMA).
            with W(17.1):
                nc.scalar.copy(rhs5[0:4, 0:256], c2_ps[:, 0:256])
                nc.vector.tensor_copy(rhs5[0:4, 256:512], c2_ps[:, 256:512])
            out_top(0, 16.9)
            out_top(1, 17.05)
            out_top(3, 17.2)
            out_top(2, 17.35)

            # ---- out += IndC^T @ [c2; b3] (normalization riding IndC),
            # 2 row-halves x 2 col-halves so the matmuls pipeline with the
            # rhs5 copies; out copies col-split across ACT/DVE ----
            H = BT // 2
            h0, h1 = slice(0, H), slice(H, BT)
            cl, cr = slice(0, 256), slice(256, 512)
            for k, (h, c) in enumerate(((h0, cl), (h0, cr), (h1, cl),
                                        (h1, cr))):
                with W(17.5 + 0.1 * k):
                    nc.tensor.matmul(out_ps[h, c], IndC[:, h], rhs5[:, c],
                                     start=False, stop=(k == 3),
                                     skip_group_check=True)
            with W(17.9):
                nc.scalar.copy(out_sb[h0, 0:256], out_ps[h0, 0:256])
                nc.vector.tensor_copy(out_sb[h0, 256:512],
                                      out_ps[h0, 256:512])
            with W(18.15):
                nc.sync.dma_start(out_d[h0, :], out_sb[h0, :])
                nc.scalar.copy(out_sb[h1, 0:256], out_ps[h1, 0:256])
                nc.vector.tensor_copy(out_sb[h1, 256:512],
                                      out_ps[h1, 256:512])
            with W(18.4):
                nc.scalar.dma_start(out_d[h1, :], out_sb[h1, :])

    nc.compile()
    return nc


def _get_graph():
    if "nc" not in _cache:
        _cache["nc"] = _build_graph()
    return _cache["nc"]


def _consts():
    if "consts" in _cache:
        return _cache["consts"]
    import ml_dtypes
    bf = ml_dtypes.bfloat16
    # mask column group s corresponds to chunk SCORD[s]
    mask = np.zeros((CH, NCH, BLOC), np.float32)
    for s in range(NCH):
        c = SCORD[s]
        for p in range(CH):
            mask[p, s, (c * CH + p) // L] = 1.0
    _cache["consts"] = {"_mask": mask.reshape(CH, MW)}
    return _cache["consts"]


def kernel(x, x_static, h0, W1, W2, W3, b2, b3, V, **_unused):
    import ml_dtypes
    from concourse.bass_utils import run_bass_kernel_spmd
    bf = ml_dtypes.bfloat16

    x = np.asarray(x, np.float32)
    x_static = np.asarray(x_static, np.float32)
    W1 = np.asarray(W1, np.float32)
    W3 = np.asarray(W3, np.float32)
    b3 = np.asarray(b3, np.float32)
    V = np.asarray(V, np.float32)

    # Host-side weight folding (weights are per-model constants).
    w1v = (W1 @ V).reshape(-1)                               # [S]
    w3t = (W3[:D].reshape(4, 128, D).transpose(1, 0, 2)
           .reshape(128, 4 * D))
    w3b = np.ascontiguousarray(
        (W3[D:] / w1v[:, None]).reshape(4, 128, D).transpose(1, 0, 2)
        .reshape(128, 4 * D).astype(bf))
    consts = _consts()
    cst = np.zeros((5, CW), np.float32)
    for b in range(BLOC):
        cst[b, b * T:(b + 1) * T] = 1.0                      # ind5 rows
    cst[4, 0:BT] = 1.0
    cst[0:4, 128:132] = np.eye(4)                            # id4
    cst[4, 132:132 + D] = b3                                 # b3 row
    cst = np.ascontiguousarray(cst.astype(bf))

    nc = _get_graph()
    in_maps = []
    for i in range(NCORES):
        sl = slice(i * BLOC, (i + 1) * BLOC)
        xsw = (x_static[sl].reshape(BL, S) * w1v[None, :])
        xs_p = xsw.reshape(NCH, CH, S).transpose(1, 0, 2)    # [CH, NCH, S]
        xsp = np.empty((CH, XSW), np.float32)
        for c in range(NCH):
            xsp[:, _CHOFF[c]:_CHOFF[c] + S] = xs_p[:, c]
        xsp[:, MASKOFF:MASKOFF + MW] = consts["_mask"]
        xsp = np.ascontiguousarray(xsp.astype(bf))
        xt_l = x[sl].reshape(BT, D).T                        # [512, 128]
        xt_p = (xt_l.reshape(4, 128, 128).transpose(1, 0, 2)
                .reshape(128, 512))
        w3tx = np.ascontiguousarray(
            np.concatenate([xt_p, w3t], axis=1).astype(bf))
        in_maps.append({
            "xsp": xsp, "w3tx": w3tx, "w3b": w3b, "cst": cst,
        })
    res = run_bass_kernel_spmd(nc, in_maps, core_ids=list(range(NCORES)))
    out = np.empty((B, T, D), np.float32)
    for i in range(NCORES):
        out[i * BLOC:(i + 1) * BLOC] = (
            res.results[i]["out"].astype(np.float32).reshape(BLOC, T, D))
    return out


# revision 19
# speedup vs baseline: 1.0071x; 1.0071x over previous
"""Bass/Trainium2 kernel for nn_Attentioncell (Bahdanau-style attention cell).

Mathematical simplification (rel-err ~6e-7 vs the jax reference): the
per-step scores are
    scores[b,l] = (total[b,l,:] + (h @ W2)[b,:]) @ V
               = (total @ V)[b,l] + (h @ W2 @ V)[b]
and softmax over l is invariant to the per-b shift, so the attention
weights are identical for every timestep and independent of h:
    attn = softmax_l(x_static @ (W1 @ V))        (b2, W2, h0 drop out)
    ctx[b,:] = sum_l attn[b,l] * x_static[b,l,:]
    out[b,t,:] = x[b,t,:] @ W3[:D] + ctx[b,:] @ W3[D:] + b3

Additional host-side weight folding: with w1v = W1@V,
    scores[l] = sum_s (x_static[l,s] * w1v[s])   = rowsum(xsw)
    ctx' = E^T @ xsw        (xsw = x_static * w1v, elementwise over s)
    ctx' @ (W3[D:] / w1v)  == ctx @ W3[D:]       (exact algebra)
so the device never multiplies by w1v: scores are plain row-sums of the
pre-scaled xsw, and W3bot is divided by w1v on the host.  Each product
term in the c2 GEMM has exactly the same magnitude as before, so the
bf16 error profile is unchanged (min |w1v| ~ 7e-5 -> max |W3bot'| ~2e3,
comfortably inside bf16 range).

Data-parallel over batch B=32 across 8 NeuronCores (4 per core).

Schedule notes (from perfetto trace analysis of the previous version):
  - PE has a DVFS p-state ramp (0.65 -> 1.2 -> 2.4 GHz after ~3us of
    continuous busy).  Dummy matmuls on a memset tile keep the PE busy
    through the ~2.6us DMA-latency head so real matmuls run at high
    clock; extra dummies are placed in known schedule bubbles.
  - 4 DMA queues (sync/scalar/gpsimd/vector), chunk order chosen by
    consumption deadline; per-DMA latency is ~2.2us fixed (descriptor
    gen + DGE start + completion-semaphore propagation) + transfer.
  - reduces: DVE tensor_reduce for chunks {0,2,4,5,6} (~.85us each),
    ACT Copy+accum for {1,3} (~1.2us each); exps in 4 waves on ACT;
    E-builds (exp * block-diagonal mask) on the otherwise-idle GpSimd.
  - Z = sum_l E via 7 tiny PE matmuls against a ones column riding the
    ctx accumulation; 1/Z is folded into the Ind5 indicator matrix
    (IndC = ind5 * recipZ) so the ctx PSUM->SBUF copy does not wait on
    the reciprocal.
  - out is produced in bf16 (halves the output DMA) and upconverted on
    the host; rel-err stays ~1e-3 vs the 2e-2 gate.
"""

import numpy as np

B, T, L, S, D = 32, 32, 196, 512, 512
NCORES = 8
BLOC = B // NCORES          # 4 batches per core
BT = BLOC * T               # 128 output rows per core
BL = BLOC * L               # 784 static rows per core
NCH = 7                     # xsw chunks
CH = BL // NCH              # 112 rows per chunk
MW = NCH * BLOC             # 28 mask columns

# xsp column layout: [c0 | c1 | mask | c2 | c3 | c4 | c5 | c6]
# (mask rides in the same DMA group as c1/c2 so it lands early for the
# first E-build).
_CHOFF = [0, S, 2 * S + MW, 3 * S + MW, 4 * S + MW, 5 * S + MW, 6 * S + MW]
MASKOFF = 2 * S
XSW = 7 * S + MW            # 3612

# score-column s <-> chunk SCORD[s]: columns ordered by expected DMA
# arrival (c5/c6 ride the GpSimd queue first DMA, c3/c4 land last) so
# reduces/exp waves run in arrival order with contiguous column slices.
SCORD = [0, 1, 2, 5, 6, 3, 4]

# consts layout: [ind5 (128) | id4 (4) | rhs5 region (512)] on 5 partitions.
# rhs5 region row 4 = b3 (via DMA); rows 0..3 overwritten on-device by c2.
CW = 128 + 4 + 512

_cache = {}


def _build_graph():
    import concourse.bacc as bacc
    import concourse.tile as tile
    from concourse import mybir

    f32 = mybir.dt.float32
    bf16 = mybir.dt.bfloat16
    nc = bacc.Bacc("TRN2", target_bir_lowering=False, debug=False,
                   num_devices=NCORES)

    xs_d = nc.dram_tensor("xsp", [CH, XSW], bf16, kind="ExternalInput").ap()
    w3t_d = nc.dram_tensor("w3tx", [128, 512 + 4 * D], bf16,
                           kind="ExternalInput").ap()
    w3b_d = nc.dram_tensor("w3b", [128, 4 * D], bf16, kind="ExternalInput").ap()
    cst_d = nc.dram_tensor("cst", [5, CW], bf16, kind="ExternalInput").ap()
    out_d = nc.dram_tensor("out", [BT, D], bf16, kind="ExternalOutput").ap()

    with tile.TileContext(nc) as tc:
        with (
            tc.tile_pool(name="big", bufs=1) as big,
            tc.tile_pool(name="small", bufs=1) as small,
            tc.tile_pool(name="scratch", bufs=2) as scratch,
            tc.tile_pool(name="ps_acc", bufs=1, space="PSUM") as ps_acc,
            tc.tile_pool(name="ps_tr", bufs=2, space="PSUM") as ps_tr,
        ):
            xsp = big.tile([CH, XSW], bf16, tag="xsp")
            mask = xsp[:, MASKOFF:MASKOFF + MW]
            w3tx = big.tile([128, 512 + 4 * D], bf16, tag="w3tx")
            xt = w3tx[:, 0:512]
            w3t = w3tx[:, 512:]
            w3b = big.tile([128, 4 * D], bf16, tag="w3b")
            cst = small.tile([5, CW], bf16, tag="cst")
            ind5 = cst[:, 0:128]
            id4 = cst[0:4, 128:132]
            rhs5 = cst[:, 132:132 + D]
            dummy = big.tile([128, 512], bf16, tag="dummy")
            ones = small.tile([CH, 1], bf16, tag="ones")
            scores = small.tile([CH, NCH], f32, tag="scores")
            etile = small.tile([CH, NCH], bf16, tag="etile")
            E = small.tile([CH, MW], bf16, tag="E")
            recipZ5 = small.tile([5, 1], f32, tag="recipZ5")
            IndC = small.tile([5, BT], bf16, tag="IndC")
            ctx_sb = small.tile([BLOC, S], bf16, tag="ctx_sb")
            ctxT = small.tile([128, 4 * BLOC], bf16, tag="ctxT")
            out_sb = big.tile([BT, D], bf16, tag="out_sb")

            # ---- memsets first (dummy feeds the PE warmup stream; on DVE
            # so the GpSimd DMA queue starts immediately) ----
            nc.vector.memset(dummy[:], 0.0)

            # ---- DMA loads. The 3 queues (SP/ACT/GpSimd) share ~300GB/s
            # of HBM and each queue processes its list serially, so the
            # global arrival order is round-based across queues: round 1-2
            # carry all score chunks, later rounds carry the GEMM weights
            # (w3b before w3t: the c2 chain consumes w3b ~1.5us before the
            # final matmuls need w3t). ----
            def chunk(c):
                return xsp[:, _CHOFF[c]:_CHOFF[c] + S]

            nc.sync.dma_start(xsp[:, 0:S], xs_d[:, 0:S])                 # c0
            nc.scalar.dma_start(xsp[:, S:_CHOFF[2] + S],
                                xs_d[:, S:_CHOFF[2] + S])                # c1+mask+c2
            nc.gpsimd.dma_start(xsp[:, _CHOFF[5]:_CHOFF[5] + 2 * S],
                                xs_d[:, _CHOFF[5]:_CHOFF[5] + 2 * S])    # c5+c6
            nc.sync.dma_start(xsp[:, _CHOFF[3]:_CHOFF[3] + S],
                              xs_d[:, _CHOFF[3]:_CHOFF[3] + S])          # c3
            nc.scalar.dma_start(xsp[:, _CHOFF[4]:_CHOFF[4] + S],
                                xs_d[:, _CHOFF[4]:_CHOFF[4] + S])        # c4
            nc.gpsimd.dma_start(cst[:], cst_d[:])                        # consts
            nc.sync.dma_start(w3b[:, 0:2 * D], w3b_d[:, 0:2 * D])        # w3b01
            nc.scalar.dma_start(w3b[:, 2 * D:], w3b_d[:, 2 * D:])        # w3b23
            nc.gpsimd.dma_start(w3tx[:, 0:1024], w3t_d[:, 0:1024])       # xt+w3t0
            nc.sync.dma_start(w3tx[:, 1024 + 2 * D:],
                              w3t_d[:, 1024 + 2 * D:])                   # w3t3
            nc.gpsimd.dma_start(w3tx[:, 1024:1024 + 2 * D],
                                w3t_d[:, 1024:1024 + 2 * D])             # w3t12
            nc.gpsimd.memset(ones[:], 1.0)
            nc.gpsimd.memset(recipZ5[:], 1.0)  # rows 0..3 overwritten by recip

            out_ps = ps_acc.tile([BT, D], f32, tag="out_ps")
            ctx_ps = ps_acc.tile([BLOC, S], f32, tag="ctx_ps")
            z_ps = ps_acc.tile([BLOC, 1], f32, tag="z_ps")
            c2_ps = ps_acc.tile([BLOC, D], f32, tag="c2_ps")
            dm_ps = ps_acc.tile([128, 512], f32, tag="dm_ps")

            # Manual schedule pins (us): the Tile scheduler's DMA model has
            # no HBM-contention term so it predicts arrivals ~3us early,
            # then compiles engine orders / semaphore bindings that stall
            # the real execution.  tile_wait_until pins each op near its
            # measured real time so the compiled per-engine order matches
            # reality; at runtime everything still runs as early as its
            # data semaphores allow.
            def W(t_us):
                return tc.tile_wait_until(t_us / 1000.0)

            # ---- PE warmup: keep the array busy through the DMA head so
            # the p-state ramps to full clock before real work arrives ----
            def dummy_mm(n):
                for i in range(n):
                    nc.tensor.matmul(dm_ps[:], dummy[:, 0:128], dummy[:],
                                     start=True, stop=True,
                                     skip_group_check=True)

            dummy_mm(8)
            # out_ps accumulation-group start: a zero contribution from the
            # memset dummy tile.  The scheduler hoists PSUM group-start
            # matmuls; giving start=True to this dep-free zero matmul keeps
            # the real out_top matmuls (which wait on the late w3t DMAs)
            # from blocking the in-order PE queue.
            nc.tensor.matmul(out_ps[:], dummy[:, 0:128], dummy[:],
                             start=True, stop=False, skip_group_check=True)
            dummy_mm(2)

            # ---- score reduces, in score-column (= arrival) order.
            # scores[:, s] = rowsum(chunk SCORD[s]); DVE does most, ACT
            # two, GpSimd pre-folds c6 into a [112,256] pair-sum. ----
            c6sum = scratch.tile([CH, 256], bf16, tag="c6sum")

            def dve_reduce(s, t):
                with W(t):
                    nc.vector.tensor_reduce(scores[:, s:s + 1],
                                            chunk(SCORD[s]),
                                            axis=mybir.AxisListType.X,
                                            op=mybir.AluOpType.add)

            def act_reduce(s, t):
                dump = scratch.tile([CH, S], bf16, tag="dump")
                with W(t):
                    nc.scalar.activation(dump[:], chunk(SCORD[s]),
                                         mybir.ActivationFunctionType.Copy,
                                         accum_out=scores[:, s:s + 1])

            def exp_wave(a, b, t):
                with W(t):
                    nc.scalar.activation(etile[:, a:b], scores[:, a:b],
                                         mybir.ActivationFunctionType.Exp)

            def ebuild(a, b, t):
                n = b - a
                with W(t):
                    nc.gpsimd.tensor_mul(
                        E[:, a * BLOC:b * BLOC].rearrange(
                            "p (c b) -> p c b", b=BLOC),
                        etile[:, a:b].to_broadcast((CH, n, BLOC)),
                        mask[:, a * BLOC:b * BLOC].rearrange(
                            "p (c b) -> p c b", b=BLOC),
                    )

            def ctx_mm(s, t):
                with W(t):
                    nc.tensor.matmul(ctx_ps[:],
                                     E[:, s * BLOC:(s + 1) * BLOC],
                                     chunk(SCORD[s]), start=(s == 0),
                                     stop=(s == NCH - 1))
                    nc.tensor.matmul(z_ps[:], E[:, s * BLOC:(s + 1) * BLOC],
                                     ones[:], start=(s == 0),
                                     stop=(s == NCH - 1))

            def out_top(j, t):
                with W(t):
                    nc.tensor.matmul(out_ps[:], xt[:, j * 128:(j + 1) * 128],
                                     w3t[:, j * D:(j + 1) * D],
                                     start=False, stop=False,
                                     skip_group_check=True)

            # reduces (score col, pin): DVE cols 0,2,3,4h,6; ACT cols 1,5
            dve_reduce(0, 9.85)
            dve_reduce(2, 10.85)
            dve_reduce(3, 12.0)
            with W(12.0):
                nc.gpsimd.tensor_add(c6sum[:], chunk(6)[:, 0:256],
                                     chunk(6)[:, 256:512])
            with W(12.75):
                nc.vector.tensor_reduce(scores[:, 4:5], c6sum[:],
                                        axis=mybir.AxisListType.X,
                                        op=mybir.AluOpType.add)
            dve_reduce(6, 13.15)
            act_reduce(1, 10.7)
            exp_wave(0, 1, 11.65)
            exp_wave(1, 3, 12.0)
            exp_wave(3, 4, 12.75)
            act_reduce(5, 13.05)
            exp_wave(4, 6, 14.1)
            exp_wave(6, 7, 14.4)
            ebuild(0, 1, 12.0)
            ebuild(1, 3, 12.3)
            ebuild(3, 4, 13.1)
            ebuild(4, 6, 14.45)
            ebuild(6, 7, 14.7)
            # PE ctx/z stream
            ctx_mm(0, 12.4)
            ctx_mm(1, 12.65)
            ctx_mm(2, 12.9)
            ctx_mm(3, 13.2)
            ctx_mm(4, 14.55)
            ctx_mm(5, 14.8)
            ctx_mm(6, 15.0)

            # ---- 1/Z -> IndC (off the ctx critical path) ----
            with W(15.35):
                nc.vector.reciprocal(recipZ5[0:4, :], z_ps[:])
            with W(15.75):
                nc.vector.tensor_scalar_mul(IndC[:], ind5[:], recipZ5[:])

            # ---- ctx PSUM->SBUF (col-split ACT/DVE), transpose, c2 GEMM ----
            with W(15.3):
                nc.scalar.copy(ctx_sb[:, 0:256], ctx_ps[:, 0:256])
            with W(15.5):
                nc.vector.tensor_copy(ctx_sb[:, 256:512], ctx_ps[:, 256:512])
            for j in range(4):
                tr = ps_tr.tile([128, BLOC], bf16, tag="tr")
                with W(15.9 + 0.08 * j):
                    nc.tensor.transpose(tr[:],
                                        ctx_sb[:, j * 128:(j + 1) * 128],
                                        id4)
                with W(16.0 + 0.12 * j):
                    nc.vector.tensor_copy(ctxT[:, j * BLOC:(j + 1) * BLOC],
                                          tr[:])
            for j in range(4):
                with W(16.3 + 0.17 * j):
                    nc.tensor.matmul(c2_ps[:],
                                     ctxT[:, j * BLOC:(j + 1) * BLOC],
                                     w3b[:, j * D:(j + 1) * D],
                                     start=(j == 0), stop=(j == 3))

            # rhs5 rows 0..3 = unnormalized c2 (split copy ACT/DVE);
            # row 4 = b3 (already there via the consts DMA).
            with W(17.05):
                nc.scalar.copy(rhs5[0:4, 0:256], c2_ps[:, 0:256])
                nc.vector.tensor_copy(rhs5[0:4, 256:512], c2_ps[:, 256:512])
            out_top(0, 16.1)
            out_top(3, 16.6)
            out_top(1, 17.35)
            out_top(2, 17.45)

            # ---- out += IndC^T @ [c2; b3] (normalization riding IndC),
            # 2 row-halves x 2 col-halves so the matmuls pipeline with the
            # rhs5 copies; out copies col-split across ACT/DVE ----
            H = BT // 2
            h0, h1 = slice(0, H), slice(H, BT)
            cl, cr = slice(0, 256), slice(256, 512)
            for k, (h, c) in enumerate(((h0, cl), (h0, cr), (h1, cl),
                                        (h1, cr))):
                with W(17.6 + 0.1 * k):
                    nc.tensor.matmul(out_ps[h, c], IndC[:, h], rhs5[:, c],
                                     start=False, stop=(k == 3),
                                     skip_group_check=True)
            with W(18.0):
                nc.scalar.copy(out_sb[h0, 0:256], out_ps[h0, 0:256])
                nc.vector.tensor_copy(out_sb[h0, 256:512],
                                      out_ps[h0, 256:512])
            with W(18.3):
                nc.sync.dma_start(out_d[h0, :], out_sb[h0, :])
                nc.scalar.copy(out_sb[h1, 0:256], out_ps[h1, 0:256])
                nc.vector.tensor_copy(out_sb[h1, 256:512],
                                      out_ps[h1, 256:512])
            with W(18.55):
                nc.scalar.dma_start(out_d[h1, :], out_sb[h1, :])

    nc.compile()
    return nc


def _get_graph():
    if "nc" not in _cache:
        _cache["nc"] = _build_graph()
    return _cache["nc"]


def _consts():
    if "consts" in _cache:
        return _cache["consts"]
    import ml_dtypes
    bf = ml_dtypes.bfloat16
    # mask column group s corresponds to chunk SCORD[s]
    mask = np.zeros((CH, NCH, BLOC), np.float32)
    for s in range(NCH):
        c = SCORD[s]
        for p in range(CH):
            mask[p, s, (c * CH + p) // L] = 1.0
    _cache["consts"] = {"_mask": mask.reshape(CH, MW)}
    return _cache["consts"]


def kernel(x, x_static, h0, W1, W2, W3, b2, b3, V, **_unused):
    import ml_dtypes
    from concourse.bass_utils import run_bass_kernel_spmd
    bf = ml_dtypes.bfloat16

    x = np.asarray(x, np.float32)
    x_static = np.asarray(x_static, np.float32)
    W1 = np.asarray(W1, np.float32)
    W3 = np.asarray(W3, np.float32)
    b3 = np.asarray(b3, np.float32)
    V = np.asarray(V, np.float32)

    # Host-side weight folding (weights are per-model constants).
    w1v = (W1 @ V).reshape(-1)                               # [S]
    w3t = (W3[:D].reshape(4, 128, D).transpose(1, 0, 2)
           .reshape(128, 4 * D))
    w3b = np.ascontiguousarray(
        (W3[D:] / w1v[:, None]).reshape(4, 128, D).transpose(1, 0, 2)
        .reshape(128, 4 * D).astype(bf))
    consts = _consts()
    cst = np.zeros((5, CW), np.float32)
    for b in range(BLOC):
        cst[b, b * T:(b + 1) * T] = 1.0                      # ind5 rows
    cst[4, 0:BT] = 1.0
    cst[0:4, 128:132] = np.eye(4)                            # id4
    cst[4, 132:132 + D] = b3                                 # b3 row
    cst = np.ascontiguousarray(cst.astype(bf))

    nc = _get_graph()
    in_maps = []
    for i in range(NCORES):
        sl = slice(i * BLOC, (i + 1) * BLOC)
        xsw = (x_static[sl].reshape(BL, S) * w1v[None, :])
        xs_p = xsw.reshape(NCH, CH, S).transpose(1, 0, 2)    # [CH, NCH, S]
        xsp = np.empty((CH, XSW), np.float32)
        for c in range(NCH):
            xsp[:, _CHOFF[c]:_CHOFF[c] + S] = xs_p[:, c]
        xsp[:, MASKOFF:MASKOFF + MW] = consts["_mask"]
        xsp = np.ascontiguousarray(xsp.astype(bf))
        xt_l = x[sl].reshape(BT, D).T                        # [512, 128]
        xt_p = (xt_l.reshape(4, 128, 128).transpose(1, 0, 2)
                .reshape(128, 512))
        w3tx = np.ascontiguousarray(
            np.concatenate([xt_p, w3t], axis=1).astype(bf))
        in_maps.append({
            "xsp": xsp, "w3tx": w3tx, "w3b": w3b, "cst": cst,
        })
    res = run_bass_kernel_spmd(nc, in_maps, core_ids=list(range(NCORES)))
    out = np.empty((B, T, D), np.float32)
    for i in range(NCORES):
        out[i * BLOC:(i + 1) * BLOC] = (
            res.results[i]["out"].astype(np.float32).reshape(BLOC, T, D))
    return out


# revision 20
# speedup vs baseline: 1.0663x; 1.0588x over previous
"""Bass/Trainium2 kernel for nn_Attentioncell (Bahdanau-style attention cell).

Mathematical simplification (rel-err ~6e-7 vs the jax reference): the
per-step scores are
    scores[b,l] = (total[b,l,:] + (h @ W2)[b,:]) @ V
               = (total @ V)[b,l] + (h @ W2 @ V)[b]
and softmax over l is invariant to the per-b shift, so the attention
weights are identical for every timestep and independent of h:
    attn = softmax_l(x_static @ (W1 @ V))        (b2, W2, h0 drop out)
    ctx[b,:] = sum_l attn[b,l] * x_static[b,l,:]
    out[b,t,:] = x[b,t,:] @ W3[:D] + ctx[b,:] @ W3[D:] + b3

Additional host-side weight folding: with w1v = W1@V,
    scores[l] = sum_s (x_static[l,s] * w1v[s])   = rowsum(xsw)
    ctx' = E^T @ xsw        (xsw = x_static * w1v, elementwise over s)
    ctx' @ (W3[D:] / w1v)  == ctx @ W3[D:]       (exact algebra)
so the device never multiplies by w1v: scores are plain row-sums of the
pre-scaled xsw, and W3bot is divided by w1v on the host.  Each product
term in the c2 GEMM has exactly the same magnitude as before, so the
bf16 error profile is unchanged (min |w1v| ~ 7e-5 -> max |W3bot'| ~2e3,
comfortably inside bf16 range).

Data-parallel over batch B=32 across 8 NeuronCores (4 per core).

Schedule notes (from perfetto trace analysis of the previous version):
  - PE has a DVFS p-state ramp (0.65 -> 1.2 -> 2.4 GHz after ~3us of
    continuous busy).  Dummy matmuls on a memset tile keep the PE busy
    through the ~2.6us DMA-latency head so real matmuls run at high
    clock; extra dummies are placed in known schedule bubbles.
  - 4 DMA queues (sync/scalar/gpsimd/vector), chunk order chosen by
    consumption deadline; per-DMA latency is ~2.2us fixed (descriptor
    gen + DGE start + completion-semaphore propagation) + transfer.
  - reduces: DVE tensor_reduce for chunks {0,2,4,5,6} (~.85us each),
    ACT Copy+accum for {1,3} (~1.2us each); exps in 4 waves on ACT;
    E-builds (exp * block-diagonal mask) on the otherwise-idle GpSimd.
  - Z = sum_l E via 7 tiny PE matmuls against a ones column riding the
    ctx accumulation; 1/Z is folded into the Ind5 indicator matrix
    (IndC = ind5 * recipZ) so the ctx PSUM->SBUF copy does not wait on
    the reciprocal.
  - out is produced in bf16 (halves the output DMA) and upconverted on
    the host; rel-err stays ~1e-3 vs the 2e-2 gate.
"""

import numpy as np

B, T, L, S, D = 32, 32, 196, 512, 512
NCORES = 8
BLOC = B // NCORES          # 4 batches per core
BT = BLOC * T               # 128 output rows per core
BL = BLOC * L               # 784 static rows per core
NCH = 7                     # xsw chunks
CH = BL // NCH              # 112 rows per chunk
MW = NCH * BLOC             # 28 mask columns

# xsp column layout: [c0 | c1 | mask | c2 | c3 | c4 | c5 | c6]
# (mask rides in the same DMA group as c1/c2 so it lands early for the
# first E-build).
_CHOFF = [0, S, 2 * S + MW, 3 * S + MW, 4 * S + MW, 5 * S + MW, 6 * S + MW]
MASKOFF = 2 * S
XSW = 7 * S + MW            # 3612

# score-column s <-> chunk SCORD[s]: columns ordered by expected DMA
# arrival (c5/c6 ride the GpSimd queue first DMA, c3/c4 land last) so
# reduces/exp waves run in arrival order with contiguous column slices.
SCORD = [0, 1, 2, 5, 6, 3, 4]

# consts layout: [ind5 (128) | id4 (4) | rhs5 region (512)] on 5 partitions.
# rhs5 region row 4 = b3 (via DMA); rows 0..3 overwritten on-device by c2.
CW = 128 + 4 + 512

_cache = {}


def _build_graph():
    import concourse.bacc as bacc
    import concourse.tile as tile
    from concourse import mybir

    f32 = mybir.dt.float32
    bf16 = mybir.dt.bfloat16
    nc = bacc.Bacc("TRN2", target_bir_lowering=False, debug=False,
                   num_devices=NCORES)

    xs_d = nc.dram_tensor("xsp", [CH, XSW], bf16, kind="ExternalInput").ap()
    w3t_d = nc.dram_tensor("w3tx", [128, 512 + 4 * D], bf16,
                           kind="ExternalInput").ap()
    w3b_d = nc.dram_tensor("w3b", [128, 4 * D], bf16, kind="ExternalInput").ap()
    cst_d = nc.dram_tensor("cst", [5, CW], bf16, kind="ExternalInput").ap()
    out_d = nc.dram_tensor("out", [BT, D], bf16, kind="ExternalOutput").ap()

    with tile.TileContext(nc) as tc:
        with (
            tc.tile_pool(name="big", bufs=1) as big,
            tc.tile_pool(name="small", bufs=1) as small,
            tc.tile_pool(name="scratch", bufs=2) as scratch,
            tc.tile_pool(name="ps_acc", bufs=1, space="PSUM") as ps_acc,
            tc.tile_pool(name="ps_tr", bufs=2, space="PSUM") as ps_tr,
        ):
            xsp = big.tile([CH, XSW], bf16, tag="xsp")
            mask = xsp[:, MASKOFF:MASKOFF + MW]
            w3tx = big.tile([128, 512 + 4 * D], bf16, tag="w3tx")
            xt = w3tx[:, 0:512]
            w3t = w3tx[:, 512:]
            w3b = big.tile([128, 4 * D], bf16, tag="w3b")
            cst = small.tile([5, CW], bf16, tag="cst")
            ind5 = cst[:, 0:128]
            id4 = cst[0:4, 128:132]
            rhs5 = cst[:, 132:132 + D]
            dummy = big.tile([128, 512], bf16, tag="dummy")
            ones = small.tile([CH, 1], bf16, tag="ones")
            scores = small.tile([CH, NCH], f32, tag="scores")
            etile = small.tile([CH, NCH], bf16, tag="etile")
            E = small.tile([CH, MW], bf16, tag="E")
            recipZ5 = small.tile([5, 1], f32, tag="recipZ5")
            IndC = small.tile([5, BT], bf16, tag="IndC")
            ctx_sb = small.tile([BLOC, S], bf16, tag="ctx_sb")
            ctxT = small.tile([128, 4 * BLOC], bf16, tag="ctxT")
            out_sb0 = big.tile([BT // 2, D], bf16, tag="out_sb0")
            out_sb1 = big.tile([BT // 2, D], bf16, tag="out_sb1")

            # ---- memsets first (dummy feeds the PE warmup stream; on DVE
            # so the GpSimd DMA queue starts immediately) ----
            nc.vector.memset(dummy[:], 0.0)

            # ---- DMA loads. The 3 queues (SP/ACT/GpSimd) share ~300GB/s
            # of HBM and each queue processes its list serially, so the
            # global arrival order is round-based across queues: round 1-2
            # carry all score chunks, later rounds carry the GEMM weights
            # (w3b before w3t: the c2 chain consumes w3b ~1.5us before the
            # final matmuls need w3t). ----
            def chunk(c):
                return xsp[:, _CHOFF[c]:_CHOFF[c] + S]

            nc.sync.dma_start(xsp[:, 0:S], xs_d[:, 0:S])                 # c0
            nc.scalar.dma_start(xsp[:, S:_CHOFF[2] + S],
                                xs_d[:, S:_CHOFF[2] + S])                # c1+mask+c2
            nc.gpsimd.dma_start(xsp[:, _CHOFF[5]:_CHOFF[5] + 2 * S],
                                xs_d[:, _CHOFF[5]:_CHOFF[5] + 2 * S])    # c5+c6
            nc.sync.dma_start(xsp[:, _CHOFF[3]:_CHOFF[3] + S],
                              xs_d[:, _CHOFF[3]:_CHOFF[3] + S])          # c3
            nc.scalar.dma_start(xsp[:, _CHOFF[4]:_CHOFF[4] + S],
                                xs_d[:, _CHOFF[4]:_CHOFF[4] + S])        # c4
            nc.gpsimd.dma_start(cst[:], cst_d[:])                        # consts
            nc.sync.dma_start(w3b[:, 0:2 * D], w3b_d[:, 0:2 * D])        # w3b01
            nc.scalar.dma_start(w3b[:, 2 * D:], w3b_d[:, 2 * D:])        # w3b23
            nc.gpsimd.dma_start(w3tx[:, 0:1024], w3t_d[:, 0:1024])       # xt+w3t0
            nc.sync.dma_start(w3tx[:, 1024 + 2 * D:],
                              w3t_d[:, 1024 + 2 * D:])                   # w3t3
            nc.gpsimd.dma_start(w3tx[:, 1024:1024 + 2 * D],
                                w3t_d[:, 1024:1024 + 2 * D])             # w3t12
            nc.gpsimd.memset(ones[:], 1.0)
            nc.gpsimd.memset(recipZ5[:], 1.0)  # rows 0..3 overwritten by recip

            out_ps = ps_acc.tile([BT, D], f32, tag="out_ps")
            ctx_ps = ps_acc.tile([BLOC, S], f32, tag="ctx_ps")
            z_ps = ps_acc.tile([BLOC, 1], f32, tag="z_ps")
            c2_ps = ps_acc.tile([BLOC, D], f32, tag="c2_ps")
            dm_ps = ps_acc.tile([128, 512], f32, tag="dm_ps")

            # Manual schedule pins (us): the Tile scheduler's DMA model has
            # no HBM-contention term so it predicts arrivals ~3us early,
            # then compiles engine orders / semaphore bindings that stall
            # the real execution.  tile_wait_until pins each op near its
            # measured real time so the compiled per-engine order matches
            # reality; at runtime everything still runs as early as its
            # data semaphores allow.
            def W(t_us):
                return tc.tile_wait_until(t_us / 1000.0)

            # ---- PE warmup: keep the array busy through the DMA head so
            # the p-state ramps to full clock before real work arrives ----
            def dummy_mm(n):
                for i in range(n):
                    nc.tensor.matmul(dm_ps[:], dummy[:, 0:128], dummy[:],
                                     start=True, stop=True,
                                     skip_group_check=True)

            dummy_mm(8)
            # out_ps accumulation-group start: a zero contribution from the
            # memset dummy tile.  The scheduler hoists PSUM group-start
            # matmuls; giving start=True to this dep-free zero matmul keeps
            # the real out_top matmuls (which wait on the late w3t DMAs)
            # from blocking the in-order PE queue.
            nc.tensor.matmul(out_ps[:], dummy[:, 0:128], dummy[:],
                             start=True, stop=False, skip_group_check=True)
            dummy_mm(2)

            # ---- score reduces, in score-column (= arrival) order.
            # scores[:, s] = rowsum(chunk SCORD[s]); DVE does most, ACT
            # two, GpSimd pre-folds c6 into a [112,256] pair-sum. ----
            c6sum = scratch.tile([CH, 256], bf16, tag="c6sum")

            def dve_reduce(s, t):
                with W(t):
                    nc.vector.tensor_reduce(scores[:, s:s + 1],
                                            chunk(SCORD[s]),
                                            axis=mybir.AxisListType.X,
                                            op=mybir.AluOpType.add)

            def act_reduce(s, t):
                dump = scratch.tile([CH, S], bf16, tag="dump")
                with W(t):
                    nc.scalar.activation(dump[:], chunk(SCORD[s]),
                                         mybir.ActivationFunctionType.Copy,
                                         accum_out=scores[:, s:s + 1])

            def exp_wave(a, b, t):
                with W(t):
                    nc.scalar.activation(etile[:, a:b], scores[:, a:b],
                                         mybir.ActivationFunctionType.Exp)

            def ebuild(a, b, t):
                n = b - a
                with W(t):
                    nc.gpsimd.tensor_mul(
                        E[:, a * BLOC:b * BLOC].rearrange(
                            "p (c b) -> p c b", b=BLOC),
                        etile[:, a:b].to_broadcast((CH, n, BLOC)),
                        mask[:, a * BLOC:b * BLOC].rearrange(
                            "p (c b) -> p c b", b=BLOC),
                    )

            def ctx_mm(s, t):
                with W(t):
                    nc.tensor.matmul(ctx_ps[:],
                                     E[:, s * BLOC:(s + 1) * BLOC],
                                     chunk(SCORD[s]), start=(s == 0),
                                     stop=(s == NCH - 1))
                    nc.tensor.matmul(z_ps[:], E[:, s * BLOC:(s + 1) * BLOC],
                                     ones[:], start=(s == 0),
                                     stop=(s == NCH - 1))

            def out_top(j, t):
                with W(t):
                    nc.tensor.matmul(out_ps[:], xt[:, j * 128:(j + 1) * 128],
                                     w3t[:, j * D:(j + 1) * D],
                                     start=False, stop=False,
                                     skip_group_check=True)

            # reduces (score col, pin): DVE cols 0,2,3,4h,6; ACT cols 1,5
            dve_reduce(0, 9.85)
            dve_reduce(2, 10.85)
            dve_reduce(3, 12.0)
            with W(12.0):
                nc.gpsimd.tensor_add(c6sum[:], chunk(6)[:, 0:256],
                                     chunk(6)[:, 256:512])
            with W(12.75):
                nc.vector.tensor_reduce(scores[:, 4:5], c6sum[:],
                                        axis=mybir.AxisListType.X,
                                        op=mybir.AluOpType.add)
            dve_reduce(6, 13.15)
            act_reduce(1, 10.7)
            exp_wave(0, 1, 11.65)
            exp_wave(1, 3, 12.0)
            exp_wave(3, 4, 12.75)
            act_reduce(5, 13.05)
            exp_wave(4, 6, 14.1)
            exp_wave(6, 7, 14.4)
            ebuild(0, 1, 12.0)
            ebuild(1, 3, 12.3)
            ebuild(3, 4, 13.1)
            ebuild(4, 6, 14.45)
            ebuild(6, 7, 14.7)
            # PE ctx/z stream
            ctx_mm(0, 12.4)
            ctx_mm(1, 12.65)
            ctx_mm(2, 12.9)
            ctx_mm(3, 13.2)
            ctx_mm(4, 14.55)
            ctx_mm(5, 14.8)
            ctx_mm(6, 15.0)

            # ---- 1/Z -> IndC (off the ctx critical path) ----
            with W(15.35):
                nc.vector.reciprocal(recipZ5[0:4, :], z_ps[:])
            with W(15.75):
                nc.vector.tensor_scalar_mul(IndC[:], ind5[:], recipZ5[:])

            # ---- ctx PSUM->SBUF (single ACT copy: a column-split pair
            # serializes anyway via tile-granular dep tracking) ----
            with W(15.2):
                nc.scalar.copy(ctx_sb[:], ctx_ps[:])
            # keep the PE p-state hot through the ctx-copy window
            for i in range(4):
                with W(15.15 + 0.2 * i):
                    nc.tensor.matmul(dm_ps[:], dummy[:, 0:128], dummy[:],
                                     start=True, stop=True,
                                     skip_group_check=True)
            for j in range(4):
                tr = ps_tr.tile([128, BLOC], bf16, tag="tr")
                with W(15.9 + 0.08 * j):
                    nc.tensor.transpose(tr[:],
                                        ctx_sb[:, j * 128:(j + 1) * 128],
                                        id4)
                with W(16.0 + 0.12 * j):
                    nc.vector.tensor_copy(ctxT[:, j * BLOC:(j + 1) * BLOC],
                                          tr[:])
            for j in range(4):
                with W(16.3 + 0.17 * j):
                    nc.tensor.matmul(c2_ps[:],
                                     ctxT[:, j * BLOC:(j + 1) * BLOC],
                                     w3b[:, j * D:(j + 1) * D],
                                     start=(j == 0), stop=(j == 3))

            # rhs5 rows 0..3 = unnormalized c2 (single DVE copy);
            # row 4 = b3 (already there via the consts DMA).
            with W(17.05):
                nc.vector.tensor_copy(rhs5[0:4, :], c2_ps[:])
            out_top(0, 17.0)
            out_top(3, 17.15)
            out_top(1, 17.3)
            out_top(2, 17.45)

            # ---- out += IndC^T @ [c2; b3] (normalization riding IndC),
            # 2 row-halves x 2 col-halves so the matmuls pipeline with the
            # rhs5 copies; out copies col-split across ACT/DVE ----
            H = BT // 2
            h0, h1 = slice(0, H), slice(H, BT)
            cl, cr = slice(0, 256), slice(256, 512)
            with W(17.75):
                nc.tensor.matmul(out_ps[h0, :], IndC[:, h0], rhs5[:],
                                 start=False, stop=False,
                                 skip_group_check=True)
            with W(17.9):
                nc.tensor.matmul(out_ps[h1, :], IndC[:, h1], rhs5[:],
                                 start=False, stop=True,
                                 skip_group_check=True)
            with W(18.05):
                nc.scalar.copy(out_sb0[:], out_ps[h0, :])
            with W(18.25):
                nc.vector.tensor_copy(out_sb1[:], out_ps[h1, :])
            with W(18.8):
                nc.sync.dma_start(out_d[h0, :], out_sb0[:])
            with W(19.05):
                nc.scalar.dma_start(out_d[h1, :], out_sb1[:])

    nc.compile()
    return nc


def _get_graph():
    if "nc" not in _cache:
        _cache["nc"] = _build_graph()
    return _cache["nc"]


def _consts():
    if "consts" in _cache:
        return _cache["consts"]
    import ml_dtypes
    bf = ml_dtypes.bfloat16
    # mask column group s corresponds to chunk SCORD[s]
    mask = np.zeros((CH, NCH, BLOC), np.float32)
    for s in range(NCH):
        c = SCORD[s]
        for p in range(CH):
            mask[p, s, (c * CH + p) // L] = 1.0
    _cache["consts"] = {"_mask": mask.reshape(CH, MW)}
    return _cache["consts"]


def kernel(x, x_static, h0, W1, W2, W3, b2, b3, V, **_unused):
    import ml_dtypes
    from concourse.bass_utils import run_bass_kernel_spmd
    bf = ml_dtypes.bfloat16

    x = np.asarray(x, np.float32)
    x_static = np.asarray(x_static, np.float32)
    W1 = np.asarray(W1, np.float32)
    W3 = np.asarray(W3, np.float32)
    b3 = np.asarray(b3, np.float32)
    V = np.asarray(V, np.float32)

    # Host-side weight folding (weights are per-model constants).
    w1v = (W1 @ V).reshape(-1)                               # [S]
    w3t = (W3[:D].reshape(4, 128, D).transpose(1, 0, 2)
           .reshape(128, 4 * D))
    w3b = np.ascontiguousarray(
        (W3[D:] / w1v[:, None]).reshape(4, 128, D).transpose(1, 0, 2)
        .reshape(128, 4 * D).astype(bf))
    consts = _consts()
    cst = np.zeros((5, CW), np.float32)
    for b in range(BLOC):
        cst[b, b * T:(b + 1) * T] = 1.0                      # ind5 rows
    cst[4, 0:BT] = 1.0
    cst[0:4, 128:132] = np.eye(4)                            # id4
    cst[4, 132:132 + D] = b3                                 # b3 row
    cst = np.ascontiguousarray(cst.astype(bf))

    nc = _get_graph()
    in_maps = []
    for i in range(NCORES):
        sl = slice(i * BLOC, (i + 1) * BLOC)
        xsw = (x_static[sl].reshape(BL, S) * w1v[None, :])
        xs_p = xsw.reshape(NCH, CH, S).transpose(1, 0, 2)    # [CH, NCH, S]
        xsp = np.empty((CH, XSW), np.float32)
        for c in range(NCH):
            xsp[:, _CHOFF[c]:_CHOFF[c] + S] = xs_p[:, c]
        xsp[:, MASKOFF:MASKOFF + MW] = consts["_mask"]
        xsp = np.ascontiguousarray(xsp.astype(bf))
        xt_l = x[sl].reshape(BT, D).T                        # [512, 128]
        xt_p = (xt_l.reshape(4, 128, 128).transpose(1, 0, 2)
                .reshape(128, 512))
        w3tx = np.ascontiguousarray(
            np.concatenate([xt_p, w3t], axis=1).astype(bf))
        in_maps.append({
            "xsp": xsp, "w3tx": w3tx, "w3b": w3b, "cst": cst,
        })
    res = run_bass_kernel_spmd(nc, in_maps, core_ids=list(range(NCORES)))
    out = np.empty((B, T, D), np.float32)
    for i in range(NCORES):
        out[i * BLOC:(i + 1) * BLOC] = (
            res.results[i]["out"].astype(np.float32).reshape(BLOC, T, D))
    return out


# revision 22
# speedup vs baseline: 1.1219x; 1.0521x over previous
"""Bass/Trainium2 kernel for nn_Attentioncell (Bahdanau-style attention cell).

Mathematical simplification (rel-err ~6e-7 vs the jax reference): the
per-step scores are
    scores[b,l] = (total[b,l,:] + (h @ W2)[b,:]) @ V
               = (total @ V)[b,l] + (h @ W2 @ V)[b]
and softmax over l is invariant to the per-b shift, so the attention
weights are identical for every timestep and independent of h:
    attn = softmax_l(x_static @ (W1 @ V))        (b2, W2, h0 drop out)
    ctx[b,:] = sum_l attn[b,l] * x_static[b,l,:]
    out[b,t,:] = x[b,t,:] @ W3[:D] + ctx[b,:] @ W3[D:] + b3

Additional host-side weight folding: with w1v = W1@V,
    scores[l] = sum_s (x_static[l,s] * w1v[s])   = rowsum(xsw)
    ctx' = E^T @ xsw        (xsw = x_static * w1v, elementwise over s)
    ctx' @ (W3[D:] / w1v)  == ctx @ W3[D:]       (exact algebra)
so the device never multiplies by w1v: scores are plain row-sums of the
pre-scaled xsw, and W3bot is divided by w1v on the host.  Each product
term in the c2 GEMM has exactly the same magnitude as before, so the
bf16 error profile is unchanged (min |w1v| ~ 7e-5 -> max |W3bot'| ~2e3,
comfortably inside bf16 range).

Data-parallel over batch B=32 across 8 NeuronCores (4 per core).

Schedule notes (from perfetto trace analysis of the previous version):
  - PE has a DVFS p-state ramp (0.65 -> 1.2 -> 2.4 GHz after ~3us of
    continuous busy).  Dummy matmuls on a memset tile keep the PE busy
    through the ~2.6us DMA-latency head so real matmuls run at high
    clock; extra dummies are placed in known schedule bubbles.
  - 4 DMA queues (sync/scalar/gpsimd/vector), chunk order chosen by
    consumption deadline; per-DMA latency is ~2.2us fixed (descriptor
    gen + DGE start + completion-semaphore propagation) + transfer.
  - reduces: DVE tensor_reduce for chunks {0,2,4,5,6} (~.85us each),
    ACT Copy+accum for {1,3} (~1.2us each); exps in 4 waves on ACT;
    E-builds (exp * block-diagonal mask) on the otherwise-idle GpSimd.
  - Z = sum_l E via 7 tiny PE matmuls against a ones column riding the
    ctx accumulation; 1/Z is folded into the Ind5 indicator matrix
    (IndC = ind5 * recipZ) so the ctx PSUM->SBUF copy does not wait on
    the reciprocal.
  - out is produced in bf16 (halves the output DMA) and upconverted on
    the host; rel-err stays ~1e-3 vs the 2e-2 gate.
"""

import numpy as np

B, T, L, S, D = 32, 32, 196, 512, 512
NCORES = 8
BLOC = B // NCORES          # 4 batches per core
BT = BLOC * T               # 128 output rows per core
BL = BLOC * L               # 784 static rows per core
NCH = 7                     # xsw chunks
CH = BL // NCH              # 112 rows per chunk
MW = NCH * BLOC             # 28 mask columns

# xsp column layout: [c0 | c1 | mask | c2 | c3 | c4 | c5 | c6]
# (mask rides in the same DMA group as c1/c2 so it lands early for the
# first E-build).
_CHOFF = [0, S, 2 * S + MW, 3 * S + MW, 4 * S + MW, 5 * S + MW, 6 * S + MW]
MASKOFF = 2 * S
XSW = 7 * S + MW            # 3612

# score-column s <-> chunk SCORD[s]: columns ordered by expected DMA
# arrival (c5/c6 ride the GpSimd queue first DMA, c3/c4 land last) so
# reduces/exp waves run in arrival order with contiguous column slices.
SCORD = [0, 1, 2, 5, 6, 3, 4]

# consts layout: [ind5 (128) | id4 (4) | rhs5 region (512)] on 5 partitions.
# rhs5 region row 4 = b3 (via DMA); rows 0..3 overwritten on-device by c2.
CW = 128 + 4 + 512

_cache = {}


def _build_graph():
    import concourse.bacc as bacc
    import concourse.tile as tile
    from concourse import mybir

    f32 = mybir.dt.float32
    bf16 = mybir.dt.bfloat16
    nc = bacc.Bacc("TRN2", target_bir_lowering=False, debug=False,
                   num_devices=NCORES)

    xs_d = nc.dram_tensor("xsp", [CH, XSW], bf16, kind="ExternalInput").ap()
    w3t_d = nc.dram_tensor("w3tx", [128, 512 + 4 * D], bf16,
                           kind="ExternalInput").ap()
    w3b_d = nc.dram_tensor("w3b", [128, 4 * D], bf16, kind="ExternalInput").ap()
    cst_d = nc.dram_tensor("cst", [5, CW], bf16, kind="ExternalInput").ap()
    out_d = nc.dram_tensor("out", [BT, D], bf16, kind="ExternalOutput").ap()

    with tile.TileContext(nc) as tc:
        with (
            tc.tile_pool(name="big", bufs=1) as big,
            tc.tile_pool(name="small", bufs=1) as small,
            tc.tile_pool(name="scratch", bufs=2) as scratch,
            tc.tile_pool(name="ps_acc", bufs=1, space="PSUM") as ps_acc,
            tc.tile_pool(name="ps_tr", bufs=4, space="PSUM") as ps_tr,
        ):
            xsp = big.tile([CH, XSW], bf16, tag="xsp")
            mask = xsp[:, MASKOFF:MASKOFF + MW]
            w3tx = big.tile([128, 512 + 4 * D], bf16, tag="w3tx")
            xt = w3tx[:, 0:512]
            w3t = w3tx[:, 512:]
            w3b = big.tile([128, 4 * D], bf16, tag="w3b")
            cst = small.tile([5, CW], bf16, tag="cst")
            ind5 = cst[:, 0:128]
            id4 = cst[0:4, 128:132]
            rhs5 = cst[:, 132:132 + D]
            dummy = big.tile([128, 512], bf16, tag="dummy")
            ones = small.tile([CH, 1], bf16, tag="ones")
            scores = small.tile([CH, NCH], f32, tag="scores")
            etile = small.tile([CH, NCH], bf16, tag="etile")
            E = small.tile([CH, MW], bf16, tag="E")
            recipZ5 = small.tile([5, 1], f32, tag="recipZ5")
            IndC = small.tile([5, BT], bf16, tag="IndC")
            ctx_sb = small.tile([BLOC, S], bf16, tag="ctx_sb")
            ctxT = small.tile([128, 4 * BLOC], bf16, tag="ctxT")
            out_sb0 = big.tile([BT // 2, D], bf16, tag="out_sb0")
            out_sb1 = big.tile([BT // 2, D], bf16, tag="out_sb1")

            # ---- memsets first (dummy feeds the PE warmup stream; on DVE
            # so the GpSimd DMA queue starts immediately) ----
            nc.vector.memset(dummy[:], 0.0)

            # ---- DMA loads. The 3 queues (SP/ACT/GpSimd) share ~300GB/s
            # of HBM and each queue processes its list serially, so the
            # global arrival order is round-based across queues: round 1-2
            # carry all score chunks, later rounds carry the GEMM weights
            # (w3b before w3t: the c2 chain consumes w3b ~1.5us before the
            # final matmuls need w3t). ----
            def chunk(c):
                return xsp[:, _CHOFF[c]:_CHOFF[c] + S]

            nc.sync.dma_start(xsp[:, 0:S], xs_d[:, 0:S])                 # c0
            nc.scalar.dma_start(xsp[:, S:_CHOFF[2] + S],
                                xs_d[:, S:_CHOFF[2] + S])                # c1+mask+c2
            nc.gpsimd.dma_start(xsp[:, _CHOFF[5]:_CHOFF[5] + 2 * S],
                                xs_d[:, _CHOFF[5]:_CHOFF[5] + 2 * S])    # c5+c6
            nc.sync.dma_start(xsp[:, _CHOFF[3]:_CHOFF[3] + S],
                              xs_d[:, _CHOFF[3]:_CHOFF[3] + S])          # c3
            nc.scalar.dma_start(xsp[:, _CHOFF[4]:_CHOFF[4] + S],
                                xs_d[:, _CHOFF[4]:_CHOFF[4] + S])        # c4
            nc.gpsimd.dma_start(cst[:], cst_d[:])                        # consts
            nc.sync.dma_start(w3b[:, 0:2 * D], w3b_d[:, 0:2 * D])        # w3b01
            nc.scalar.dma_start(w3b[:, 2 * D:], w3b_d[:, 2 * D:])        # w3b23
            nc.gpsimd.dma_start(w3tx[:, 0:1024], w3t_d[:, 0:1024])       # xt+w3t0
            nc.sync.dma_start(w3tx[:, 1024 + 2 * D:],
                              w3t_d[:, 1024 + 2 * D:])                   # w3t3
            nc.gpsimd.dma_start(w3tx[:, 1024:1024 + 2 * D],
                                w3t_d[:, 1024:1024 + 2 * D])             # w3t12
            nc.gpsimd.memset(ones[:], 1.0)
            nc.gpsimd.memset(recipZ5[:], 1.0)  # rows 0..3 overwritten by recip

            out_ps = ps_acc.tile([BT, D], f32, tag="out_ps")
            ctx_ps = ps_acc.tile([BLOC, S], f32, tag="ctx_ps")
            z_ps = ps_acc.tile([BLOC, 1], f32, tag="z_ps")
            c2_ps = ps_acc.tile([BLOC, D], f32, tag="c2_ps")

            # Manual schedule pins (us): the Tile scheduler's DMA model has
            # no HBM-contention term so it predicts arrivals ~3us early,
            # then compiles engine orders / semaphore bindings that stall
            # the real execution.  tile_wait_until pins each op near its
            # measured real time so the compiled per-engine order matches
            # reality; at runtime everything still runs as early as its
            # data semaphores allow.
            def W(t_us):
                return tc.tile_wait_until(t_us / 1000.0)

            # ---- PE warmup: keep the array busy through the DMA head so
            # the p-state ramps to full clock before real work arrives ----
            def dummy_mm(n):
                for i in range(n):
                    nc.tensor.matmul(c2_ps[:], dummy[:, 0:BLOC], dummy[:],
                                     start=True, stop=True,
                                     skip_group_check=True)

            dummy_mm(8)
            # out_ps accumulation-group start: a zero contribution from the
            # memset dummy tile.  The scheduler hoists PSUM group-start
            # matmuls; giving start=True to this dep-free zero matmul keeps
            # the real out_top matmuls (which wait on the late w3t DMAs)
            # from blocking the in-order PE queue.
            nc.tensor.matmul(out_ps[:], dummy[:, 0:128], dummy[:],
                             start=True, stop=False, skip_group_check=True)
            dummy_mm(2)

            # ---- score reduces, in score-column (= arrival) order.
            # scores[:, s] = rowsum(chunk SCORD[s]); DVE does most, ACT
            # two, GpSimd pre-folds c6 into a [112,256] pair-sum. ----
            c6sum = scratch.tile([CH, 256], bf16, tag="c6sum")

            def dve_reduce(s, t):
                with W(t):
                    nc.vector.tensor_reduce(scores[:, s:s + 1],
                                            chunk(SCORD[s]),
                                            axis=mybir.AxisListType.X,
                                            op=mybir.AluOpType.add)

            def act_reduce(s, t):
                dump = scratch.tile([CH, S], bf16, tag="dump")
                with W(t):
                    nc.scalar.activation(dump[:], chunk(SCORD[s]),
                                         mybir.ActivationFunctionType.Copy,
                                         accum_out=scores[:, s:s + 1])

            def exp_wave(a, b, t):
                with W(t):
                    nc.scalar.activation(etile[:, a:b], scores[:, a:b],
                                         mybir.ActivationFunctionType.Exp)

            def ebuild(a, b, t):
                n = b - a
                with W(t):
                    nc.gpsimd.tensor_mul(
                        E[:, a * BLOC:b * BLOC].rearrange(
                            "p (c b) -> p c b", b=BLOC),
                        etile[:, a:b].to_broadcast((CH, n, BLOC)),
                        mask[:, a * BLOC:b * BLOC].rearrange(
                            "p (c b) -> p c b", b=BLOC),
                    )

            def ctx_mm(s, t):
                with W(t):
                    nc.tensor.matmul(ctx_ps[:],
                                     E[:, s * BLOC:(s + 1) * BLOC],
                                     chunk(SCORD[s]), start=(s == 0),
                                     stop=(s == NCH - 1))
                    nc.tensor.matmul(z_ps[:], E[:, s * BLOC:(s + 1) * BLOC],
                                     ones[:], start=(s == 0),
                                     stop=(s == NCH - 1))

            def out_top(j, t):
                with W(t):
                    nc.tensor.matmul(out_ps[:], xt[:, j * 128:(j + 1) * 128],
                                     w3t[:, j * D:(j + 1) * D],
                                     start=False, stop=False,
                                     skip_group_check=True)

            # reduces (score col, pin): DVE cols 0,2,3,4h,6; ACT cols 1,5
            dve_reduce(0, 9.85)
            dve_reduce(2, 10.85)
            dve_reduce(3, 12.0)
            with W(12.0):
                nc.gpsimd.tensor_add(c6sum[:], chunk(6)[:, 0:256],
                                     chunk(6)[:, 256:512])
            with W(12.75):
                nc.vector.tensor_reduce(scores[:, 4:5], c6sum[:],
                                        axis=mybir.AxisListType.X,
                                        op=mybir.AluOpType.add)
            dve_reduce(6, 13.15)
            act_reduce(1, 10.7)
            exp_wave(0, 1, 11.65)
            exp_wave(1, 3, 12.0)
            exp_wave(3, 4, 12.75)
            act_reduce(5, 13.05)
            exp_wave(4, 6, 14.1)
            exp_wave(6, 7, 14.4)
            ebuild(0, 1, 12.0)
            ebuild(1, 3, 12.3)
            ebuild(3, 4, 13.1)
            ebuild(4, 6, 14.45)
            ebuild(6, 7, 14.7)
            # PE ctx/z stream
            ctx_mm(0, 12.4)
            ctx_mm(1, 12.65)
            ctx_mm(2, 12.9)
            ctx_mm(3, 13.2)
            ctx_mm(4, 14.55)
            ctx_mm(5, 14.8)
            ctx_mm(6, 15.0)

            # ---- 1/Z -> IndC (off the ctx critical path) ----
            with W(15.35):
                nc.vector.reciprocal(recipZ5[0:4, :], z_ps[:])
            with W(15.75):
                nc.vector.tensor_scalar_mul(IndC[:], ind5[:], recipZ5[:])

            # ---- ctx PSUM->SBUF (single ACT copy: a column-split pair
            # serializes anyway via tile-granular dep tracking) ----
            with W(15.2):
                nc.scalar.copy(ctx_sb[:], ctx_ps[:])
            # keep the PE p-state hot through the ctx-copy window
            for i in range(4):
                with W(15.15 + 0.2 * i):
                    nc.tensor.matmul(c2_ps[:], dummy[:, 0:BLOC], dummy[:],
                                     start=True, stop=True,
                                     skip_group_check=True)
            for j in range(4):
                tr = ps_tr.tile([128, BLOC], bf16, tag="tr")
                with W(15.9 + 0.08 * j):
                    nc.tensor.transpose(tr[:],
                                        ctx_sb[:, j * 128:(j + 1) * 128],
                                        id4)
                with W(16.0 + 0.12 * j):
                    nc.vector.tensor_copy(ctxT[:, j * BLOC:(j + 1) * BLOC],
                                          tr[:])
            for j in range(4):
                with W(16.3 + 0.17 * j):
                    nc.tensor.matmul(c2_ps[:],
                                     ctxT[:, j * BLOC:(j + 1) * BLOC],
                                     w3b[:, j * D:(j + 1) * D],
                                     start=(j == 0), stop=(j == 3))

            # rhs5 rows 0..3 = unnormalized c2 (single ACT copy; ACT is
            # idle between the ctx copy and the out copy).
            # row 4 = b3 (already there via the consts DMA).
            with W(17.7):
                nc.scalar.copy(rhs5[0:4, :], c2_ps[:])
            out_top(0, 17.85)
            out_top(3, 18.0)
            out_top(1, 18.15)
            out_top(2, 18.3)

            # ---- out += IndC^T @ [c2; b3] (normalization riding IndC),
            # 2 row-halves x 2 col-halves so the matmuls pipeline with the
            # rhs5 copies; out copies col-split across ACT/DVE ----
            H = BT // 2
            h0, h1 = slice(0, H), slice(H, BT)
            cl, cr = slice(0, 256), slice(256, 512)
            with W(18.5):
                nc.tensor.matmul(out_ps[h0, :], IndC[:, h0], rhs5[:],
                                 start=False, stop=True,
                                 skip_group_check=True)
            with W(18.65):
                nc.tensor.matmul(out_ps[h1, :], IndC[:, h1], rhs5[:],
                                 start=False, stop=True,
                                 skip_group_check=True)
            with W(18.8):
                nc.scalar.copy(out_sb0[:], out_ps[h0, :])
            with W(18.85):
                nc.vector.tensor_copy(out_sb1[:], out_ps[h1, :])
            with W(19.5):
                nc.sync.dma_start(out_d[h0, :], out_sb0[:])
            with W(19.6):
                nc.scalar.dma_start(out_d[h1, :], out_sb1[:])

    nc.compile()
    return nc


def _get_graph():
    if "nc" not in _cache:
        _cache["nc"] = _build_graph()
    return _cache["nc"]


def _consts():
    if "consts" in _cache:
        return _cache["consts"]
    import ml_dtypes
    bf = ml_dtypes.bfloat16
    # mask column group s corresponds to chunk SCORD[s]
    mask = np.zeros((CH, NCH, BLOC), np.float32)
    for s in range(NCH):
        c = SCORD[s]
        for p in range(CH):
            mask[p, s, (c * CH + p) // L] = 1.0
    _cache["consts"] = {"_mask": mask.reshape(CH, MW)}
    return _cache["consts"]


def kernel(x, x_static, h0, W1, W2, W3, b2, b3, V, **_unused):
    import ml_dtypes
    from concourse.bass_utils import run_bass_kernel_spmd
    bf = ml_dtypes.bfloat16

    x = np.asarray(x, np.float32)
    x_static = np.asarray(x_static, np.float32)
    W1 = np.asarray(W1, np.float32)
    W3 = np.asarray(W3, np.float32)
    b3 = np.asarray(b3, np.float32)
    V = np.asarray(V, np.float32)

    # Host-side weight folding (weights are per-model constants).
    w1v = (W1 @ V).reshape(-1)                               # [S]
    w3t = (W3[:D].reshape(4, 128, D).transpose(1, 0, 2)
           .reshape(128, 4 * D))
    w3b = np.ascontiguousarray(
        (W3[D:] / w1v[:, None]).reshape(4, 128, D).transpose(1, 0, 2)
        .reshape(128, 4 * D).astype(bf))
    consts = _consts()
    cst = np.zeros((5, CW), np.float32)
    for b in range(BLOC):
        cst[b, b * T:(b + 1) * T] = 1.0                      # ind5 rows
    cst[4, 0:BT] = 1.0
    cst[0:4, 128:132] = np.eye(4)                            # id4
    cst[4, 132:132 + D] = b3                                 # b3 row
    cst = np.ascontiguousarray(cst.astype(bf))

    nc = _get_graph()
    in_maps = []
    for i in range(NCORES):
        sl = slice(i * BLOC, (i + 1) * BLOC)
        xsw = (x_static[sl].reshape(BL, S) * w1v[None, :])
        xs_p = xsw.reshape(NCH, CH, S).transpose(1, 0, 2)    # [CH, NCH, S]
        xsp = np.empty((CH, XSW), np.float32)
        for c in range(NCH):
            xsp[:, _CHOFF[c]:_CHOFF[c] + S] = xs_p[:, c]
        xsp[:, MASKOFF:MASKOFF + MW] = consts["_mask"]
        xsp = np.ascontiguousarray(xsp.astype(bf))
        xt_l = x[sl].reshape(BT, D).T                        # [512, 128]
        xt_p = (xt_l.reshape(4, 128, 128).transpose(1, 0, 2)
                .reshape(128, 512))
        w3tx = np.ascontiguousarray(
            np.concatenate([xt_p, w3t], axis=1).astype(bf))
        in_maps.append({
            "xsp": xsp, "w3tx": w3tx, "w3b": w3b, "cst": cst,
        })
    res = run_bass_kernel_spmd(nc, in_maps, core_ids=list(range(NCORES)))
    out = np.empty((B, T, D), np.float32)
    for i in range(NCORES):
        out[i * BLOC:(i + 1) * BLOC] = (
            res.results[i]["out"].astype(np.float32).reshape(BLOC, T, D))
    return out
